# revision 10
# baseline (speedup 1.0000x reference)
"""Trainium2 Bass kernel for nn_AxonalConnections (gnn_message_passing).

Computes out[b,t] = sum_s adjacency[t,s] * mod[b,s],  mod = (1.5*E - 0.5) * spikes,
i.e. a batched mat-vec against a [16384, 16384] adjacency, reshaped to [32,128,128].

Sharding: adjacency row-shard (target dim) across 8 cores; spikes/E replicated;
each core produces out[:, t_shard] — pure output sharding, no collectives.

Two device paths:

* dense: bf16 GEMM, K=16384 accumulated in fp32 PSUM (fallback for arbitrary
  adjacency).

* sparse: when the adjacency's nonzeros all lie on the 9 conv-pattern
  diagonals (the generator's 3x3 message-passing graph), the GEMM is exactly a
  9-tap locally-connected stencil: out[b,t] = sum_k w9[t,k]*mod[b,t+d_k],
  evaluated on a [4 t-quarters x 32 batch, 512] packed layout (E-modulation
  folded into the weights on the host — exact, the factor is {1.0, -0.5}).

  v4 (all fp16, pure DVE pipeline):
  - The whole stencil runs as 9 tensor_tensor mults + 8 adds in fp16 so the
    DVE hits its 2x_1P mode (~420 ns per [128,512] op, measured). Tap
    offsets have mixed parity, so the padded spike slab ships in both
    parities (S0/S1, one element apart) and each tap reads whichever copy
    makes its window start 4-byte aligned.
  - Profiling showed the body is gated by DMA *landing latency*, not DMA
    bytes: a tensor's completion semaphore fires only when the last SDMA
    engine finishes it, so big monolithic transfers (and few-partition
    tensors, which serialize on a single engine) delay the first compute by
    ~5 us. So every tap's batch-replicated weights ship as their own
    [128, 512] DMA, split across the two HWDGE rings (SP + ACT) in exactly
    the order the DVE consumes them; the DVE starts ~2.5 us earlier and
    never starves.
  - The final add runs in two halves, each immediately followed by its own
    output DMA, so the two HBM-write receipts overlap the tail instead of
    serializing after the last add.
"""

import sys

if "/opt/trn_rl_repo" not in sys.path:
    sys.path.insert(0, "/opt/trn_rl_repo")

from contextlib import ExitStack

import ml_dtypes
import numpy as np

B = 32
H = 128
W = 128
S = H * W            # 16384
NCORES = 8
TL = S // NCORES     # 2048 t-columns per core
KC = S // 128        # 128 contraction chunks (dense path)
P = 128

# sparse path geometry: 3x3 conv neighborhood offsets in flattened index space
DIAG_OFFSETS = [di * W + dj for di in (-1, 0, 1) for dj in (-1, 0, 1)]
NTAP = len(DIAG_OFFSETS)
NQ = 4               # t-quarters packed on partitions: 4*32 = 128
QT = TL // NQ        # 512 t per quarter
PADE = 130           # left pad of the spike slab
SW = 776             # per-parity slab width (max window start 258 + 512, even)
# S0[i] = sp[tq + i - PADE] -> tap d at even offset 130+d for even d
# S1[i] = S0[i+1]           -> tap d at even offset 129+d for odd d
# DVE consumption order: even-d taps first (S0 lands first), then the rest.
# Each HWDGE ring drains FIFO at ~150 GB/s (measured), so weight tensors ship
# mostly as per-tap singles, striped across the two rings in consumption
# order — each tap's completion semaphore fires just before the DVE needs it.
# The tail taps pair up to limit total DMA count (8 completion-sem lanes).
TAP_ORDER = [1, 4, 7, 0, 3, 6, 2, 5, 8]
WGROUPS = [(1,), (4,), (7,), (0,), (3,), (2,), (6,), (5, 8)]
WG_RING = ["act", "act", "act", "act", "act", "act", "sync", "sync"]

_progs = {}


def _build_dense():
    import concourse.tile as tile
    from concourse import bacc, mybir

    nc = bacc.Bacc("TRN2", target_bir_lowering=False, debug=False, num_devices=NCORES)
    f32 = mybir.dt.float32
    bf16 = mybir.dt.bfloat16

    adjt = nc.dram_tensor("adjt", [S, TL], bf16, kind="ExternalInput").ap()
    spt = nc.dram_tensor("spt", [P, KC, B], f32, kind="ExternalInput").ap()
    ef = nc.dram_tensor("ef", [P, KC], f32, kind="ExternalInput").ap()
    outt = nc.dram_tensor("out", [B, TL], f32, kind="ExternalOutput").ap()

    NT = TL // 512  # psum banks used for the output row block

    with tile.TileContext(nc) as tc:
        with ExitStack() as ctx:
            const = ctx.enter_context(tc.tile_pool(name="const", bufs=1))
            adj_pool = ctx.enter_context(tc.tile_pool(name="adj", bufs=10))
            psum = ctx.enter_context(tc.tile_pool(name="psum", bufs=1, space="PSUM"))
            outp = ctx.enter_context(tc.tile_pool(name="outp", bufs=1))

            sp_t = const.tile([P, KC, B], f32)
            nc.sync.dma_start(sp_t[:], spt[:])
            e_t = const.tile([P, KC], f32)
            nc.sync.dma_start(e_t[:], ef[:])
            fac = const.tile([P, KC], f32)
            # fac = 1.5*E - 0.5  (E in {0,1} -> {1.0, -0.5})
            nc.vector.tensor_scalar(
                fac[:], e_t[:], 1.5, -0.5,
                op0=mybir.AluOpType.mult, op1=mybir.AluOpType.add,
            )
            modt = const.tile([P, KC, B], bf16)
            for k in range(KC):
                nc.vector.tensor_scalar(
                    modt[:, k, :], sp_t[:, k, :], fac[:, k : k + 1], None,
                    op0=mybir.AluOpType.mult,
                )

            pts = [psum.tile([B, 512], f32, name=f"acc{j}") for j in range(NT)]
            for k in range(KC):
                at = adj_pool.tile([P, TL], bf16)
                nc.sync.dma_start(at[:], adjt[k * P : (k + 1) * P, :])
                for j in range(NT):
                    nc.tensor.matmul(
                        pts[j][:],
                        modt[:, k, :],
                        at[:, j * 512 : (j + 1) * 512],
                        start=(k == 0),
                        stop=(k == KC - 1),
                    )

            ot = outp.tile([B, TL], f32)
            for j in range(NT):
                nc.vector.tensor_copy(out=ot[:, j * 512 : (j + 1) * 512], in_=pts[j][:])
            nc.sync.dma_start(outt[:], ot[:])

    nc.compile()
    return nc


def _tap_slice(s0, s1, d):
    """AP slice of the dual-parity spike slabs for tap offset d (start even)."""
    if d % 2 == 0:
        return s0[:, 130 + d : 130 + d + QT]
    return s1[:, 129 + d : 129 + d + QT]


def _build_sparse():
    import concourse.tile as tile
    from concourse import bacc, mybir

    nc = bacc.Bacc("TRN2", target_bir_lowering=False, debug=False, num_devices=NCORES)
    f16 = mybir.dt.float16

    s0d = nc.dram_tensor("s0", [P, SW], f16, kind="ExternalInput").ap()
    s1d = nc.dram_tensor("s1", [P, SW], f16, kind="ExternalInput").ap()
    wgd = [
        nc.dram_tensor(f"wg{g}", [P, len(ks), QT], f16, kind="ExternalInput").ap()
        for g, ks in enumerate(WGROUPS)
    ]
    # packed [32q+b, t] layout; host unpacks to [B, TL]
    outt = nc.dram_tensor("out", [P, QT], f16, kind="ExternalOutput").ap()

    HQ = QT // 2

    with tile.TileContext(nc) as tc:
        with ExitStack() as ctx:
            pool = ctx.enter_context(tc.tile_pool(name="pool", bufs=1))

            # SP ring: the spike slabs then alternate weight groups; ACT ring
            # (drains in parallel) the other groups — striped in DVE
            # consumption order so each group's completion semaphore fires
            # just before its taps are consumed.
            s0 = pool.tile([P, SW], f16)
            nc.sync.dma_start(s0[:], s0d[:])
            s1 = pool.tile([P, SW], f16)
            nc.sync.dma_start(s1[:], s1d[:])
            wt = {}
            for g, ks in enumerate(WGROUPS):
                wg = pool.tile([P, len(ks), QT], f16, name=f"wg{g}")
                eng = nc.scalar if WG_RING[g] == "act" else nc.sync
                eng.dma_start(wg[:], wgd[g][:])
                for j, k in enumerate(ks):
                    wt[k] = wg[:, j, :]

            mult = mybir.AluOpType.mult
            add = mybir.AluOpType.add
            acc = None
            for i, k in enumerate(TAP_ORDER):
                d = DIAG_OFFSETS[k]
                sh = _tap_slice(s0, s1, d)
                prod = pool.tile([P, QT], f16, name=f"prod{k}")
                nc.vector.tensor_tensor(prod[:], sh, wt[k], mult)
                if acc is None:
                    acc = prod
                elif i < NTAP - 1:
                    nxt = pool.tile([P, QT], f16, name=f"acc{i}")
                    nc.vector.tensor_tensor(nxt[:], acc[:], prod[:], add)
                    acc = nxt
                else:
                    # final add + store in halves so the two HBM-write
                    # receipts overlap
                    fin = pool.tile([P, QT], f16, name="fin")
                    for h in range(2):
                        lo, hi = h * HQ, (h + 1) * HQ
                        nc.vector.tensor_tensor(
                            fin[:, lo:hi], acc[:, lo:hi], prod[:, lo:hi], add
                        )
                        nc.sync.dma_start(outt[:, lo:hi], fin[:, lo:hi])

    nc.compile()
    return nc


def _get_prog(name):
    if name not in _progs:
        _progs[name] = {"dense": _build_dense, "sparse": _build_sparse}[name]()
    return _progs[name]


def _run(nc, in_maps, **kwargs):
    from concourse.bass_utils import run_bass_kernel_spmd

    return run_bass_kernel_spmd(nc, in_maps, core_ids=list(range(NCORES)), **kwargs)


def _extract_diagonals(adjacency):
    """W9[t, k] = adjacency[t, t + d_k] (0 where out of range).

    Returns (W9, exact) where exact means every nonzero of adjacency lies on
    those 9 diagonals, making the stencil reproduction of the GEMM exact.
    """
    t = np.arange(S)
    W9 = np.zeros((S, NTAP), np.float32)
    for k, d in enumerate(DIAG_OFFSETS):
        s = t + d
        valid = (s >= 0) & (s < S)
        W9[valid, k] = adjacency[t[valid], s[valid]]
    exact = np.count_nonzero(adjacency) == np.count_nonzero(W9)
    return W9, exact


def _prep_dense_inmaps(sp_flat, E_flat, adjacency):
    spt = np.ascontiguousarray(sp_flat.T.reshape(KC, P, B).transpose(1, 0, 2))
    ef = np.ascontiguousarray(E_flat.reshape(KC, P).T)
    adj_bf = adjacency.astype(ml_dtypes.bfloat16)
    in_maps = []
    for m in range(NCORES):
        adjt_m = np.ascontiguousarray(adj_bf[m * TL : (m + 1) * TL, :].T)
        in_maps.append({"adjt": adjt_m, "spt": spt, "ef": ef})
    return in_maps


def _prep_sparse_inmaps(sp_flat, E_flat, W9):
    # fold the E-modulation into the tap weights: exact because the factor is
    # the power-of-two scale {1.0, -0.5}
    fac = 1.5 * E_flat - 0.5
    t = np.arange(S)
    wfold = np.empty_like(W9)  # [S, 9]
    for k, d in enumerate(DIAG_OFFSETS):
        s = np.clip(t + d, 0, S - 1)
        wfold[:, k] = W9[:, k] * fac[s]
    wfold = wfold.astype(np.float16)

    sp_pad = np.zeros((B, S + 2 * PADE + 8), np.float16)
    sp_pad[:, PADE : PADE + S] = sp_flat

    in_maps = []
    for m in range(NCORES):
        t0 = m * TL
        s0 = np.empty((NQ, B, SW), np.float16)
        s1 = np.empty((NQ, B, SW), np.float16)
        for q in range(NQ):
            tq = t0 + q * QT
            s0[q] = sp_pad[:, tq : tq + SW]
            s1[q] = sp_pad[:, tq + 1 : tq + 1 + SW]

        wslab = wfold[t0 : t0 + TL].reshape(NQ, QT, NTAP)
        im = {"s0": s0.reshape(P, SW), "s1": s1.reshape(P, SW)}
        for g, ks in enumerate(WGROUPS):
            wg = wslab[:, :, list(ks)].transpose(0, 2, 1)      # [NQ, |ks|, QT]
            wg = np.broadcast_to(wg[:, None], (NQ, B, len(ks), QT))
            im[f"wg{g}"] = np.ascontiguousarray(wg).reshape(P, len(ks), QT)
        in_maps.append(im)
    return in_maps


def _gather_out(results):
    out = np.empty((B, S), np.float32)
    for m in range(NCORES):
        r = results[m]["out"]
        if r.shape == (P, QT):  # sparse path: unpack [32q+b, t] -> [b, q*QT+t]
            r = r.astype(np.float32).reshape(NQ, B, QT).transpose(1, 0, 2)
            r = r.reshape(B, TL)
        out[:, m * TL : (m + 1) * TL] = r
    return out


def kernel(spikes, E, adjacency):
    spikes = np.asarray(spikes, np.float32)
    E = np.asarray(E, np.float32)
    adjacency = np.asarray(adjacency, np.float32)
    sp_flat = spikes.reshape(B, S)
    E_flat = E.reshape(S)

    W9, exact = _extract_diagonals(adjacency)
    if exact:
        in_maps = _prep_sparse_inmaps(sp_flat, E_flat, W9)
        results = _run(_get_prog("sparse"), in_maps).results
    else:
        in_maps = _prep_dense_inmaps(sp_flat, E_flat, adjacency)
        results = _run(_get_prog("dense"), in_maps).results
    return _gather_out(results).reshape(B, H, W)


# revision 15
# speedup vs baseline: 1.0317x; 1.0317x over previous
"""Trainium2 Bass kernel for nn_AxonalConnections (gnn_message_passing).

Computes out[b,t] = sum_s adjacency[t,s] * mod[b,s],  mod = (1.5*E - 0.5) * spikes,
i.e. a batched mat-vec against a [16384, 16384] adjacency, reshaped to [32,128,128].

Sharding: adjacency row-shard (target dim) across 8 cores; spikes/E replicated;
each core produces out[:, t_shard] — pure output sharding, no collectives.

Two device paths:

* dense: bf16 GEMM, K=16384 accumulated in fp32 PSUM (fallback for arbitrary
  adjacency).

* sparse: when the adjacency's nonzeros all lie on the 9 conv-pattern
  diagonals (the generator's 3x3 message-passing graph), the GEMM is exactly a
  9-tap locally-connected stencil: out[b,t] = sum_k w9[t,k]*mod[b,t+d_k],
  evaluated on a [4 t-quarters x 32 batch, 512] packed layout (E-modulation
  folded into the weights on the host — exact, the factor is {1.0, -0.5}).

  v4 (all fp16, pure DVE pipeline):
  - The whole stencil runs as 9 tensor_tensor mults + 8 adds in fp16 so the
    DVE hits its 2x_1P mode (~420 ns per [128,512] op, measured). Tap
    offsets have mixed parity, so the padded spike slab ships in both
    parities (S0/S1, one element apart) and each tap reads whichever copy
    makes its window start 4-byte aligned.
  - Profiling showed the body is gated by DMA *landing latency*, not DMA
    bytes: a tensor's completion semaphore fires only when the last SDMA
    engine finishes it, so big monolithic transfers (and few-partition
    tensors, which serialize on a single engine) delay the first compute by
    ~5 us. So every tap's batch-replicated weights ship as their own
    [128, 512] DMA, split across the two HWDGE rings (SP + ACT) in exactly
    the order the DVE consumes them; the DVE starts ~2.5 us earlier and
    never starves.
  - The final add runs in two halves, each immediately followed by its own
    output DMA, so the two HBM-write receipts overlap the tail instead of
    serializing after the last add.
"""

import sys

if "/opt/trn_rl_repo" not in sys.path:
    sys.path.insert(0, "/opt/trn_rl_repo")

from contextlib import ExitStack

import ml_dtypes
import numpy as np

B = 32
H = 128
W = 128
S = H * W            # 16384
NCORES = 8
TL = S // NCORES     # 2048 t-columns per core
KC = S // 128        # 128 contraction chunks (dense path)
P = 128

# sparse path geometry: 3x3 conv neighborhood offsets in flattened index space
DIAG_OFFSETS = [di * W + dj for di in (-1, 0, 1) for dj in (-1, 0, 1)]
NTAP = len(DIAG_OFFSETS)
NQ = 4               # t-quarters packed on partitions: 4*32 = 128
QT = TL // NQ        # 512 t per quarter
PADE = 130           # left pad of the spike slab
SW = 776             # per-parity slab width (max window start 258 + 512, even)
# S0[i] = sp[tq + i - PADE] -> tap d at even offset 130+d for even d
# S1[i] = S0[i+1]           -> tap d at even offset 129+d for odd d
# DVE consumption order: even-d taps first (S0 lands first), then the rest.
# Measured DMA law: with several transfers queued, completions smear to
# ~(first-packet + total_bytes/210GB/s); only each HWDGE ring's FIRST tensor
# lands early. So total DMA bytes are minimized: 5 taps ship COMPACT
# ([4, 512] rows + 0/1 selector, 13 KB, first on the ACT ring) and are
# batch-broadcast on-chip by TensorE (selector matmul, exact) + ScalarE
# PSUM->fp16 copies; only the last-consumed 4 taps ship batch-replicated
# (two pair tensors on the SP ring behind the slabs).
TAP_ORDER = [1, 4, 7, 0, 3, 6, 2, 5, 8]
BC_TAPS = [1, 4, 7, 0, 3]      # broadcast on-chip, in consumption order
WGROUPS = [(6, 2), (5, 8)]     # DMA'd batch-replicated pair tensors
WG_RING = ["sync", "sync"]
WCW = len(BC_TAPS) * QT + P    # compact rows: 5 taps + selector block

_progs = {}


def _build_dense():
    import concourse.tile as tile
    from concourse import bacc, mybir

    nc = bacc.Bacc("TRN2", target_bir_lowering=False, debug=False, num_devices=NCORES)
    f32 = mybir.dt.float32
    bf16 = mybir.dt.bfloat16

    adjt = nc.dram_tensor("adjt", [S, TL], bf16, kind="ExternalInput").ap()
    spt = nc.dram_tensor("spt", [P, KC, B], f32, kind="ExternalInput").ap()
    ef = nc.dram_tensor("ef", [P, KC], f32, kind="ExternalInput").ap()
    outt = nc.dram_tensor("out", [B, TL], f32, kind="ExternalOutput").ap()

    NT = TL // 512  # psum banks used for the output row block

    with tile.TileContext(nc) as tc:
        with ExitStack() as ctx:
            const = ctx.enter_context(tc.tile_pool(name="const", bufs=1))
            adj_pool = ctx.enter_context(tc.tile_pool(name="adj", bufs=10))
            psum = ctx.enter_context(tc.tile_pool(name="psum", bufs=1, space="PSUM"))
            outp = ctx.enter_context(tc.tile_pool(name="outp", bufs=1))

            sp_t = const.tile([P, KC, B], f32)
            nc.sync.dma_start(sp_t[:], spt[:])
            e_t = const.tile([P, KC], f32)
            nc.sync.dma_start(e_t[:], ef[:])
            fac = const.tile([P, KC], f32)
            # fac = 1.5*E - 0.5  (E in {0,1} -> {1.0, -0.5})
            nc.vector.tensor_scalar(
                fac[:], e_t[:], 1.5, -0.5,
                op0=mybir.AluOpType.mult, op1=mybir.AluOpType.add,
            )
            modt = const.tile([P, KC, B], bf16)
            for k in range(KC):
                nc.vector.tensor_scalar(
                    modt[:, k, :], sp_t[:, k, :], fac[:, k : k + 1], None,
                    op0=mybir.AluOpType.mult,
                )

            pts = [psum.tile([B, 512], f32, name=f"acc{j}") for j in range(NT)]
            for k in range(KC):
                at = adj_pool.tile([P, TL], bf16)
                nc.sync.dma_start(at[:], adjt[k * P : (k + 1) * P, :])
                for j in range(NT):
                    nc.tensor.matmul(
                        pts[j][:],
                        modt[:, k, :],
                        at[:, j * 512 : (j + 1) * 512],
                        start=(k == 0),
                        stop=(k == KC - 1),
                    )

            ot = outp.tile([B, TL], f32)
            for j in range(NT):
                nc.vector.tensor_copy(out=ot[:, j * 512 : (j + 1) * 512], in_=pts[j][:])
            nc.sync.dma_start(outt[:], ot[:])

    nc.compile()
    return nc


def _tap_slice(s0, s1, d):
    """AP slice of the dual-parity spike slabs for tap offset d (start even)."""
    if d % 2 == 0:
        return s0[:, 130 + d : 130 + d + QT]
    return s1[:, 129 + d : 129 + d + QT]


def _build_sparse():
    import concourse.tile as tile
    from concourse import bacc, mybir

    nc = bacc.Bacc("TRN2", target_bir_lowering=False, debug=False, num_devices=NCORES)
    f16 = mybir.dt.float16
    f32 = mybir.dt.float32

    s0d = nc.dram_tensor("s0", [P, SW], f16, kind="ExternalInput").ap()
    s1d = nc.dram_tensor("s1", [P, SW], f16, kind="ExternalInput").ap()
    wcsd = nc.dram_tensor("wcs", [4, WCW], f16, kind="ExternalInput").ap()
    wgd = [
        nc.dram_tensor(f"wg{g}", [P, len(ks), QT], f16, kind="ExternalInput").ap()
        for g, ks in enumerate(WGROUPS)
    ]
    # packed [32q+b, t] layout; host unpacks to [B, TL]
    outt = nc.dram_tensor("out", [P, QT], f16, kind="ExternalOutput").ap()

    HQ = QT // 2

    with tile.TileContext(nc) as tc:
        with ExitStack() as ctx:
            pool = ctx.enter_context(tc.tile_pool(name="pool", bufs=1))
            psum = ctx.enter_context(tc.tile_pool(name="psum", bufs=1, space="PSUM"))

            # ACT ring: the tiny compact block first (early-FIFO slot, feeds
            # the broadcast lane). SP ring: slabs, then the replicated pairs.
            wcs = pool.tile([4, WCW], f16)
            nc.scalar.dma_start(wcs[:], wcsd[:])
            s0 = pool.tile([P, SW], f16)
            nc.sync.dma_start(s0[:], s0d[:])
            s1 = pool.tile([P, SW], f16)
            nc.sync.dma_start(s1[:], s1d[:])
            wt = {}
            for g, ks in enumerate(WGROUPS):
                wg = pool.tile([P, len(ks), QT], f16, name=f"wg{g}")
                eng = nc.scalar if WG_RING[g] == "act" else nc.sync
                eng.dma_start(wg[:], wgd[g][:])
                for j, k in enumerate(ks):
                    wt[k] = wg[:, j, :]

            # broadcast lane: psum[p, t] = sum_q sel[q, p] * wcs[q, j*QT+t]
            sel = wcs[:, len(BC_TAPS) * QT : len(BC_TAPS) * QT + P]
            for j, k in enumerate(BC_TAPS):
                ps = psum.tile([P, QT], f32, name=f"bc{j}")
                nc.tensor.matmul(
                    ps[:], sel, wcs[:, j * QT : (j + 1) * QT],
                    start=True, stop=True,
                )
                wk = pool.tile([P, QT], f16, name=f"wb{k}")
                nc.scalar.copy(wk[:], ps[:])
                wt[k] = wk[:]

            mult = mybir.AluOpType.mult
            add = mybir.AluOpType.add
            acc = None
            for i, k in enumerate(TAP_ORDER):
                d = DIAG_OFFSETS[k]
                sh = _tap_slice(s0, s1, d)
                prod = pool.tile([P, QT], f16, name=f"prod{k}")
                nc.vector.tensor_tensor(prod[:], sh, wt[k], mult)
                if acc is None:
                    acc = prod
                elif i < NTAP - 1:
                    nxt = pool.tile([P, QT], f16, name=f"acc{i}")
                    nc.vector.tensor_tensor(nxt[:], acc[:], prod[:], add)
                    acc = nxt
                else:
                    # final add + store in halves so the two HBM-write
                    # receipts overlap
                    fin = pool.tile([P, QT], f16, name="fin")
                    for h in range(2):
                        lo, hi = h * HQ, (h + 1) * HQ
                        nc.vector.tensor_tensor(
                            fin[:, lo:hi], acc[:, lo:hi], prod[:, lo:hi], add
                        )
                        nc.sync.dma_start(outt[:, lo:hi], fin[:, lo:hi])

    nc.compile()
    return nc


def _get_prog(name):
    if name not in _progs:
        _progs[name] = {"dense": _build_dense, "sparse": _build_sparse}[name]()
    return _progs[name]


def _run(nc, in_maps, **kwargs):
    from concourse.bass_utils import run_bass_kernel_spmd

    return run_bass_kernel_spmd(nc, in_maps, core_ids=list(range(NCORES)), **kwargs)


def _extract_diagonals(adjacency):
    """W9[t, k] = adjacency[t, t + d_k] (0 where out of range).

    Returns (W9, exact) where exact means every nonzero of adjacency lies on
    those 9 diagonals, making the stencil reproduction of the GEMM exact.
    """
    t = np.arange(S)
    W9 = np.zeros((S, NTAP), np.float32)
    for k, d in enumerate(DIAG_OFFSETS):
        s = t + d
        valid = (s >= 0) & (s < S)
        W9[valid, k] = adjacency[t[valid], s[valid]]
    exact = np.count_nonzero(adjacency) == np.count_nonzero(W9)
    return W9, exact


def _prep_dense_inmaps(sp_flat, E_flat, adjacency):
    spt = np.ascontiguousarray(sp_flat.T.reshape(KC, P, B).transpose(1, 0, 2))
    ef = np.ascontiguousarray(E_flat.reshape(KC, P).T)
    adj_bf = adjacency.astype(ml_dtypes.bfloat16)
    in_maps = []
    for m in range(NCORES):
        adjt_m = np.ascontiguousarray(adj_bf[m * TL : (m + 1) * TL, :].T)
        in_maps.append({"adjt": adjt_m, "spt": spt, "ef": ef})
    return in_maps


def _prep_sparse_inmaps(sp_flat, E_flat, W9):
    # fold the E-modulation into the tap weights: exact because the factor is
    # the power-of-two scale {1.0, -0.5}
    fac = 1.5 * E_flat - 0.5
    t = np.arange(S)
    wfold = np.empty_like(W9)  # [S, 9]
    for k, d in enumerate(DIAG_OFFSETS):
        s = np.clip(t + d, 0, S - 1)
        wfold[:, k] = W9[:, k] * fac[s]
    wfold = wfold.astype(np.float16)

    sp_pad = np.zeros((B, S + 2 * PADE + 8), np.float16)
    sp_pad[:, PADE : PADE + S] = sp_flat

    # 0/1 selector shared across cores: sel[q, p] = (p // 32 == q)
    sel = (np.arange(P)[None, :] // B == np.arange(NQ)[:, None]).astype(np.float16)

    in_maps = []
    for m in range(NCORES):
        t0 = m * TL
        s0 = np.empty((NQ, B, SW), np.float16)
        s1 = np.empty((NQ, B, SW), np.float16)
        for q in range(NQ):
            tq = t0 + q * QT
            s0[q] = sp_pad[:, tq : tq + SW]
            s1[q] = sp_pad[:, tq + 1 : tq + 1 + SW]

        wslab = wfold[t0 : t0 + TL].reshape(NQ, QT, NTAP)
        im = {"s0": s0.reshape(P, SW), "s1": s1.reshape(P, SW)}
        wcs = np.empty((NQ, WCW), np.float16)
        wcs[:, : len(BC_TAPS) * QT] = (
            wslab[:, :, BC_TAPS].transpose(0, 2, 1).reshape(NQ, len(BC_TAPS) * QT)
        )
        wcs[:, len(BC_TAPS) * QT :] = sel
        im["wcs"] = wcs
        for g, ks in enumerate(WGROUPS):
            wg = wslab[:, :, list(ks)].transpose(0, 2, 1)      # [NQ, |ks|, QT]
            wg = np.broadcast_to(wg[:, None], (NQ, B, len(ks), QT))
            im[f"wg{g}"] = np.ascontiguousarray(wg).reshape(P, len(ks), QT)
        in_maps.append(im)
    return in_maps


def _gather_out(results):
    out = np.empty((B, S), np.float32)
    for m in range(NCORES):
        r = results[m]["out"]
        if r.shape == (P, QT):  # sparse path: unpack [32q+b, t] -> [b, q*QT+t]
            r = r.astype(np.float32).reshape(NQ, B, QT).transpose(1, 0, 2)
            r = r.reshape(B, TL)
        out[:, m * TL : (m + 1) * TL] = r
    return out


def kernel(spikes, E, adjacency):
    spikes = np.asarray(spikes, np.float32)
    E = np.asarray(E, np.float32)
    adjacency = np.asarray(adjacency, np.float32)
    sp_flat = spikes.reshape(B, S)
    E_flat = E.reshape(S)

    W9, exact = _extract_diagonals(adjacency)
    if exact:
        in_maps = _prep_sparse_inmaps(sp_flat, E_flat, W9)
        results = _run(_get_prog("sparse"), in_maps).results
    else:
        in_maps = _prep_dense_inmaps(sp_flat, E_flat, adjacency)
        results = _run(_get_prog("dense"), in_maps).results
    return _gather_out(results).reshape(B, H, W)
